# revision 1
# baseline (speedup 1.0000x reference)
"""PointPillarsScatter Trainium2 kernel.

Reference op:
  canvas[b*NY*NX + y*NX + x] = voxel_features[p]        (scatter-set, 64 ch)
  out[:, :64]  = canvas -> [B, 64, NY, NX]
  out[:, 64:]  = transpose(map_fm, (0, 3, 2, 1))        (16 ch)

Strategy (8 NeuronCores, SPMD):
  core = batch*2 + y_half  (4 batches x 2 halves of NY=496 -> NYH=248 rows).

  Scatter is computed as a one-hot matmul on the TensorEngine:
    out[128ch', 512cells] = featT[128slots, 128ch'].T @ S[128slots, 512]
  where S[s, n] = (pos[s] == n) is built on the VectorEngine with
  iota + is_equal, and ch' packs the 64 channels of TWO 512-cell tiles
  (tile j -> psum partitions 0:64, tile j+105 -> 64:128, so each
  partition half maps to a contiguous DRAM range).  This fuses
  zero-fill + scatter + transpose into one PE op per 1024 cells.

  The matmul runs in fp32r mode (1 column/cycle vs 4 for fp32). fp32r is
  e8m11 (lossy), so each feature value v is split exactly into
  v = hi + lo, both parts e8m11-representable (hi = mantissa truncated
  to 11 bits, lo = v - hi has <= 12 significant bits). hi parts occupy
  slots 0..63, lo parts slots 64..127 with identical S rows, so a single
  K=128 matmul reconstructs v exactly in fp32 PSUM.

  map_fm is transposed with PE transpose (identity) in [<=128, 128]
  blocks, staged per output y-row so every DMA moves 1728B contiguous
  runs.

Host side only computes index tables + shards inputs (per the sharding
hint: route points by coords to their core); all FP math runs on device.
"""

import sys

for _p in ("/opt/trn_rl_repo",):
    if _p not in sys.path:
        sys.path.insert(0, _p)

import numpy as np

# problem constants (hardcoded per contract)
B, NPTS, C, NY, NX, CM = 4, 48000, 64, 496, 432, 16
NYH = NY // 2            # 248 rows per core
NCORE = 8
NCELL = NYH * NX         # 107136 cells per core
TILE = 512               # cells per channel-block
PAIR = 2 * TILE          # cells per matmul (two 64ch blocks -> M=128)
NT = (NCELL + TILE - 1) // TILE          # 210 tiles (last has 128 cells)
NP = (NT + 1) // 2                       # 105 pairs: tile j with tile j+NP
ACELL = NP * TILE                        # 53760 cells in the A half
BCELL = NCELL - ACELL                    # 53376 cells in the B half
CAP = 64                 # points per pair-column (x2 slots for hi/lo)
FB = 16                  # pair-columns per feature DMA load
SG = 8                   # pairs per SBUF staging buffer / out DMA
YB = 8                   # map y rows per transpose block ( YB*CM = 128 )
NYB = NYH // YB          # 31 y-blocks
XCH = [(0, 128), (128, 128), (256, 128), (384, 48)]   # x chunks of NX=432

_prog_cache = {}


def _build_program(ncols, chunks):
    """Build the SPMD Bass program (identical for all 8 cores)."""
    from concourse import bacc, mybir, tile
    from concourse.masks import make_identity

    f32 = mybir.dt.float32
    f32r = mybir.dt.float32r
    i32 = mybir.dt.int32

    nc = bacc.Bacc(trn_type="TRN2", target_bir_lowering=False)

    # slot-major layout: partition s reads one contiguous 4KB run per load
    feat_d = nc.dram_tensor("feat", [2 * CAP, ncols * 2 * C], f32r,
                            kind="ExternalInput")
    post_d = nc.dram_tensor("post", [2 * CAP, ncols], f32, kind="ExternalInput")
    map_d = nc.dram_tensor("mapin", [NX, NYH, CM], f32, kind="ExternalInput")
    out_d = nc.dram_tensor("out", [C + CM, NCELL], f32, kind="ExternalOutput")

    # column index of each (pair, chunk)
    colbase = np.concatenate([[0], np.cumsum(chunks)]).astype(np.int64)

    # map work: one transpose+store group per y-block
    map_groups = list(range(NYB))
    mg_iter = iter(map_groups)
    n_sc_groups = (NP + SG - 1) // SG

    with tile.TileContext(nc) as tc:
        with (
            tc.tile_pool(name="const", bufs=1) as cpool,
            tc.tile_pool(name="fpool", bufs=2) as fpool,
            tc.tile_pool(name="spool", bufs=4) as spool,
            tc.tile_pool(name="stg", bufs=2) as stpool,
            tc.tile_pool(name="mstg", bufs=3) as mstpool,
            tc.tile_pool(name="mtin", bufs=1) as mtpool,
            tc.tile_pool(name="pscat", bufs=4, space="PSUM") as pspool,
            tc.tile_pool(name="pmap", bufs=3, space="PSUM") as pmpool,
        ):
            # constants
            iota_i = cpool.tile([2 * CAP, TILE], i32)
            nc.gpsimd.iota(iota_i[:], pattern=[[1, TILE]], base=0,
                           channel_multiplier=0)
            iota_f = cpool.tile([2 * CAP, TILE], f32)
            nc.gpsimd.tensor_copy(iota_f[:], iota_i[:])
            ident = cpool.tile([128, 128], f32)
            make_identity(nc, ident[:])
            posT = cpool.tile([2 * CAP, ncols], f32)
            nc.sync.dma_start(out=posT[:], in_=post_d[:])

            # preload the whole map input: 4 DMAs with 15.5KB runs
            mts = []
            for x0, w in XCH:
                mt = mtpool.tile([128, NYB * YB * CM], f32, tag="mt%d" % x0)
                nc.scalar.dma_start(
                    out=mt[:w, :],
                    in_=map_d[x0:x0 + w, :, :].rearrange("x y c -> x (y c)"))
                mts.append(mt)
            out_map = out_d[C:C + CM, :].rearrange("c (yy xx) -> c yy xx",
                                                   xx=NX)

            def emit_map_group(k):
                pm = pmpool.tile([128, NX], f32)
                for xi, (x0, w) in enumerate(XCH):
                    nc.tensor.transpose(
                        out=pm[:, x0:x0 + w],
                        in_=mts[xi][:w, (k * YB * CM):((k + 1) * YB * CM)],
                        identity=ident[:w, :w])
                ms = mstpool.tile([128, NX], f32)
                nc.scalar.copy(out=ms[:], in_=pm[:])
                # one DMA per y-block: runs of NX*4 = 1728B
                dst = out_map[:, k * YB:(k + 1) * YB, :]
                nc.scalar.dma_start(
                    out=dst.rearrange("c dy x -> dy c x"), in_=ms[:])

            # scatter loop over groups of SG pairs
            emitted_maps = 0
            for g in range(n_sc_groups):
                p0 = g * SG
                p1 = min(p0 + SG, NP)
                c0 = int(colbase[p0])
                c1 = int(colbase[p1])
                fb = fpool.tile([2 * CAP, FB * 2 * C], f32r, tag="fb")
                assert c1 - c0 <= FB, (c0, c1)
                nc.sync.dma_start(
                    out=fb[:, :(c1 - c0) * 2 * C],
                    in_=feat_d[:, c0 * 2 * C:c1 * 2 * C])
                stg = stpool.tile([128, SG * TILE], f32)
                for pr in range(p0, p1):
                    # block A = tile pr (always 512 cells),
                    # block B = tile NP+pr (last one has 128 cells)
                    nbb = min(TILE, max(0, NCELL - (NP + pr) * TILE))
                    ps = pspool.tile([128, TILE], f32)
                    nck = int(chunks[pr])
                    for k in range(nck):
                        col = int(colbase[pr]) + k
                        s_t = spool.tile([2 * CAP, TILE], f32r)
                        nc.vector.tensor_scalar(
                            out=s_t[:], in0=iota_f[:],
                            scalar1=posT[:, col:col + 1], scalar2=None,
                            op0=mybir.AluOpType.is_equal)
                        lhs = fb[:, (col - c0) * 2 * C:(col - c0 + 1) * 2 * C]
                        nc.tensor.matmul(
                            out=ps[:], lhsT=lhs,
                            rhs=s_t[:],
                            start=(k == 0), stop=(k == nck - 1))
                    off = (pr - p0) * TILE
                    eng = nc.vector.tensor_copy if pr % 5 == 0 else (
                        lambda out, in_: nc.scalar.copy(out=out, in_=in_))
                    eng(out=stg[:, off:off + TILE], in_=ps[:])
                # two out DMAs: A half + B half, each contiguous (<=8KB runs)
                wa = (p1 - p0) * TILE
                a0 = p0 * TILE
                nc.sync.dma_start(out=out_d[0:C, a0:a0 + wa],
                                  in_=stg[:64, :wa])
                wb = min(BCELL - a0, wa)
                if wb > 0:
                    nc.sync.dma_start(
                        out=out_d[0:C, ACELL + a0:ACELL + a0 + wb],
                        in_=stg[64:, :wb])
                while (emitted_maps < len(map_groups)
                       and emitted_maps * n_sc_groups <= (g + 1) * NYB):
                    emit_map_group(next(mg_iter))
                    emitted_maps += 1
            for mg in mg_iter:
                emit_map_group(mg)

    nc.finalize()
    return nc


def _host_prep(voxel_features, coords, map_fm):
    """Shard points by core, build feature/pos tables (host index work only)."""
    vf = np.ascontiguousarray(np.asarray(voxel_features), dtype=np.float32)
    cd = np.asarray(coords)
    mf = np.asarray(map_fm)
    if mf.ndim == 5:
        mf = np.squeeze(mf, 3)
    mf = np.ascontiguousarray(mf, dtype=np.float32)

    b = cd[:, 0].astype(np.int64)
    y = cd[:, 2].astype(np.int64)
    x = cd[:, 3].astype(np.int64)
    valid = (b >= 0) & (b < B) & (y >= 0) & (y < NY) & (x >= 0) & (x < NX)
    b, y, x = b[valid], y[valid], x[valid]
    vfv = vf[valid]

    half = (y >= NYH).astype(np.int64)
    core = b * 2 + half
    lcell = (y - half * NYH) * NX + x
    t = lcell // TILE          # 512-cell tile id
    pos = lcell - t * TILE     # position within tile (= matmul column)
    pair = t % NP              # tile j pairs with tile j+NP
    blk = t // NP              # channel block within the pair

    key = core * NP + pair
    order = np.argsort(key, kind="stable")
    ks = key[order]
    counts = np.bincount(ks, minlength=NCORE * NP)
    kmax = counts.reshape(NCORE, NP).max(axis=0)
    chunks = np.maximum((kmax + CAP - 1) // CAP, 1)
    for g in range(0, NP, SG):
        need = int(chunks[g:g + SG].sum())
        if need > FB:
            raise ValueError("pair group needs %d cols > FB=%d" % (need, FB))
    ncols = int(chunks.sum())
    colbase = np.concatenate([[0], np.cumsum(chunks)]).astype(np.int64)

    starts = np.concatenate([[0], np.cumsum(counts)]).astype(np.int64)
    rank = np.arange(len(ks), dtype=np.int64) - starts[ks]

    co = core[order]
    po = pair[order]
    bo = blk[order]
    colo = colbase[po] + rank // CAP
    slot = rank % CAP

    # exact fp32 = hi + lo split, both parts e8m11 (fp32r) representable
    vo = vfv[order]
    hi = (vo.view(np.uint32) & np.uint32(0xFFFFF000)).view(np.float32)
    lo = vo - hi

    feat = np.zeros((NCORE, 2 * CAP, ncols, 2 * C), np.float32)
    post = np.full((NCORE, 2 * CAP, ncols), -1.0, np.float32)
    ccol = bo[:, None] * C + np.arange(C)[None, :]
    feat[co[:, None], slot[:, None], colo[:, None], ccol] = hi
    feat[co[:, None], (CAP + slot)[:, None], colo[:, None], ccol] = lo
    posf = pos[order].astype(np.float32)
    post[co, slot, colo] = posf
    post[co, CAP + slot, colo] = posf

    maps = []
    for core_id in range(NCORE):
        bb, hh = core_id // 2, core_id % 2
        maps.append(np.ascontiguousarray(
            mf[bb, :, hh * NYH:(hh + 1) * NYH, :]))
    return feat, post, maps, ncols, chunks


def kernel(voxel_features, coords, batch_size=None, map_fm=None,
           trace=False, _return_results=False):
    from concourse.bass_utils import run_bass_kernel_spmd

    feat, post, maps, ncols, chunks = _host_prep(
        voxel_features, coords, map_fm)

    ckey = (ncols, tuple(int(c) for c in chunks))
    if ckey not in _prog_cache:
        _prog_cache.clear()
        _prog_cache[ckey] = _build_program(ncols, chunks)
    nc = _prog_cache[ckey]

    in_maps = [
        {"feat": feat[i].reshape(2 * CAP, -1), "post": post[i],
         "mapin": maps[i]}
        for i in range(NCORE)
    ]
    res = run_bass_kernel_spmd(nc, in_maps, list(range(NCORE)), trace=trace)

    out = np.empty((B, C + CM, NY, NX), np.float32)
    for core_id in range(NCORE):
        bb, hh = core_id // 2, core_id % 2
        out[bb, :, hh * NYH:(hh + 1) * NYH, :] = (
            res.results[core_id]["out"].reshape(C + CM, NYH, NX))
    if _return_results:
        return out, res
    return out



# revision 7
# speedup vs baseline: 2.6064x; 2.6064x over previous
"""PointPillarsScatter Trainium2 kernel (fp16 pipeline).

Reference op:
  canvas[b*NY*NX + y*NX + x] = voxel_features[p]        (scatter-set, 64 ch)
  out[:, :64]  = canvas -> [B, 64, NY, NX]
  out[:, 64:]  = transpose(map_fm, (0, 3, 2, 1))        (16 ch)

Strategy (8 NeuronCores, SPMD, data-parallel per sharding hint):
  core = batch*2 + y_half  (4 batches x 2 halves of NY=496 -> NYH=248 rows).

  Everything on-device runs in fp16 (correctness gate is rel_err < 2e-2;
  fp16 gives ~5e-4), which halves HBM traffic vs fp32 and lets the PE run
  at 1 column/cycle.  The scatter is a one-hot matmul on the TensorEngine:
    psum[128ch', 512cells] = feat[128slots, 128ch'].T @ S[128slots, 512]
  where S[s, n] = (pos[s] == n) is built on the VectorEngine with
  iota + is_equal (all-fp16 -> 2x DVE mode), and ch' packs the 64 channels
  of TWO 512-cell tiles (tile j -> psum partitions 0:64, tile j+105 ->
  64:128).  This fuses zero-fill + scatter + transpose into one PE op per
  1024 cells.  With fp16 there is no hi/lo split: all 128 slots hold
  points (CAP=128 per column).

  The canvas DRAM layout is [128, ACELL] fp16 (partition = half*64 + ch),
  so every store is one full-128-partition DMA with 16KB+ runs; the host
  reassembles the [64, NCELL] canvas from the two halves.

  map_fm is transposed with PE transpose (fp16 identity) in [<=128, 128]
  blocks directly into fp16 PSUM, copied 2 y-blocks at a time, and stored
  4 y-blocks per DMA (3456B contiguous runs).

Host side only computes index tables + shards/casts inputs; all FP math
(scatter + transpose) runs on device.
"""

import sys

for _p in ("/opt/trn_rl_repo",):
    if _p not in sys.path:
        sys.path.insert(0, _p)

import numpy as np

# problem constants (hardcoded per contract)
B, NPTS, C, NY, NX, CM = 4, 48000, 64, 496, 432, 16
NYH = NY // 2            # 248 rows per core
NCORE = 8
NCELL = NYH * NX         # 107136 cells per core
TILE = 512               # cells per channel-block
NT = (NCELL + TILE - 1) // TILE          # 210 tiles (last has 128 cells)
NP = (NT + 1) // 2                       # 105 pairs: tile j with tile j+NP
ACELL = NP * TILE                        # 53760 cells in the A half
BCELL = NCELL - ACELL                    # 53376 cells in the B half
CAP = 128                # point slots per column (fp16: no hi/lo split)
SG = 16                  # pairs per canvas-store group (105 = 6*16 + 9)
NSG = (NP + SG - 1) // SG                # 7 store groups
FBMAX = 24               # max feat columns per group load
YB = 8                   # map y rows per transpose block ( YB*CM = 128 )
NYB = NYH // YB          # 31 y-blocks
NMU = (NYB + 1) // 2     # 16 map units (2 y-blocks each; last has 1)
MBG = 4                  # map y-blocks per store DMA (8 stores)
XCH = [(0, 128), (128, 128), (256, 128), (384, 48)]   # x chunks of NX=432

_prog_cache = {}


def _build_program(ncols, chunks):
    """Build the SPMD Bass program (identical for all 8 cores)."""
    from concourse import bacc, mybir, tile
    from concourse.masks import make_identity

    f16 = mybir.dt.float16
    f32 = mybir.dt.float32
    i32 = mybir.dt.int32

    nc = bacc.Bacc(trn_type="TRN2", target_bir_lowering=False)

    # slot-major layout: partition s reads one contiguous run per load
    feat_d = nc.dram_tensor("feat", [CAP, ncols * 2 * C], f16,
                            kind="ExternalInput")
    post_d = nc.dram_tensor("post", [CAP, ncols], f32, kind="ExternalInput")
    map_d = nc.dram_tensor("mapin", [NX, NYH * CM], f16, kind="ExternalInput")
    canv_d = nc.dram_tensor("canv", [128, ACELL], f16, kind="ExternalOutput")
    mapo_d = nc.dram_tensor("mapo", [128, NYB * NX], f16,
                            kind="ExternalOutput")

    colbase = np.concatenate([[0], np.cumsum(chunks)]).astype(np.int64)

    with tile.TileContext(nc) as tc:
        with (
            tc.tile_pool(name="const", bufs=1) as cpool,
            tc.tile_pool(name="fpool", bufs=NSG) as fpool,
            tc.tile_pool(name="spool", bufs=4) as spool,
            tc.tile_pool(name="stg", bufs=2) as stpool,
            tc.tile_pool(name="mstg", bufs=2) as mstpool,
            tc.tile_pool(name="mtin", bufs=1) as mtpool,
            tc.tile_pool(name="pscat", bufs=3, space="PSUM") as pspool,
            tc.tile_pool(name="pmap", bufs=2, space="PSUM") as pmpool,
        ):
            # constants
            iota_i = cpool.tile([128, TILE], i32)
            nc.gpsimd.iota(iota_i[:], pattern=[[1, TILE]], base=0,
                           channel_multiplier=0)
            iota_f32 = cpool.tile([128, TILE], f32)
            nc.gpsimd.tensor_copy(iota_f32[:], iota_i[:])
            iota_f = cpool.tile([128, TILE], f16)
            nc.vector.tensor_copy(iota_f[:], iota_f32[:])
            ident = cpool.tile([128, 128], f16)
            make_identity(nc, ident[:])
            posT = cpool.tile([CAP, ncols], f32)
            nc.scalar.dma_start(out=posT[:], in_=post_d[:])

            # preload the whole map input (x-chunked to <=128 partitions)
            mts = []
            for x0, w in XCH:
                mt = mtpool.tile([128, NYH * CM], f16, tag="mt%d" % x0)
                nc.scalar.dma_start(out=mt[:w, :], in_=map_d[x0:x0 + w, :])
                mts.append(mt)

            # preload all feat column groups (removes head-of-line blocking)
            fbs = []
            for g in range(NSG):
                p0, p1 = g * SG, min((g + 1) * SG, NP)
                c0, c1 = int(colbase[p0]), int(colbase[p1])
                assert c1 - c0 <= FBMAX, (c0, c1)
                fb = fpool.tile([CAP, FBMAX * 2 * C], f16)
                nc.sync.dma_start(out=fb[:, :(c1 - c0) * 2 * C],
                                  in_=feat_d[:, c0 * 2 * C:c1 * 2 * C])
                fbs.append((fb, c0, c1))

            # ---- map transpose machinery ----
            # unit k2 covers y-blocks 2*k2, 2*k2+1 (last unit: 1 block)
            mstate = {"ms": None, "ncopy": 0}

            def emit_map_unit(k2):
                nb = 2 if 2 * k2 + 1 < NYB else 1
                pm = pmpool.tile([128, 2 * NX], f16)
                for j in range(nb):
                    kb = 2 * k2 + j
                    for xi, (x0, w) in enumerate(XCH):
                        nc.tensor.transpose(
                            out=pm[:, j * NX + x0:j * NX + x0 + w],
                            in_=mts[xi][:w, kb * 128:(kb + 1) * 128],
                            identity=ident[:w, :w])
                if k2 % 2 == 0:
                    mstate["ms"] = mstpool.tile([128, MBG * NX], f16,
                                                name="ms")
                ms = mstate["ms"]
                off = (k2 % 2) * 2 * NX
                nc.vector.tensor_copy(out=ms[:, off:off + nb * NX],
                                      in_=pm[:, :nb * NX])
                if k2 % 2 == 1 or k2 == NMU - 1:
                    blk0 = (k2 - k2 % 2) * 2
                    wm = (min(blk0 + MBG, NYB) - blk0) * NX
                    nc.sync.dma_start(
                        out=mapo_d[:, blk0 * NX:blk0 * NX + wm],
                        in_=ms[:, :wm])

            # ---- scatter main loop ----
            emitted_units = 0
            for g in range(NSG):
                p0, p1 = g * SG, min((g + 1) * SG, NP)
                fb, c0, c1 = fbs[g]
                stg = stpool.tile([128, SG * TILE], f16)
                pr = p0
                ncopy = 0
                while pr < p1:
                    npair = min(2, p1 - pr)
                    ps = pspool.tile([128, 2 * TILE], f32)
                    for q in range(npair):
                        pcur = pr + q
                        nck = int(chunks[pcur])
                        for k in range(nck):
                            col = int(colbase[pcur]) + k
                            s_t = spool.tile([128, TILE], f16)
                            nc.vector.tensor_scalar(
                                out=s_t[:], in0=iota_f[:],
                                scalar1=posT[:, col:col + 1], scalar2=None,
                                op0=mybir.AluOpType.is_equal)
                            nc.tensor.matmul(
                                out=ps[:, q * TILE:(q + 1) * TILE],
                                lhsT=fb[:, (col - c0) * 2 * C:
                                        (col - c0 + 1) * 2 * C],
                                rhs=s_t[:],
                                start=(k == 0), stop=(k == nck - 1))
                    off = (pr - p0) * TILE
                    # most copies on ACT; every 4th on DVE for balance
                    if ncopy % 4 == 3:
                        nc.vector.tensor_copy(
                            out=stg[:, off:off + npair * TILE],
                            in_=ps[:, :npair * TILE])
                    else:
                        nc.scalar.copy(
                            out=stg[:, off:off + npair * TILE],
                            in_=ps[:, :npair * TILE])
                    ncopy += 1
                    pr += npair
                a0 = p0 * TILE
                wa = (p1 - p0) * TILE
                nc.sync.dma_start(out=canv_d[:, a0:a0 + wa],
                                  in_=stg[:, :wa])
                while (emitted_units < NMU
                       and emitted_units * NSG < (g + 1) * NMU):
                    emit_map_unit(emitted_units)
                    emitted_units += 1
            while emitted_units < NMU:
                emit_map_unit(emitted_units)
                emitted_units += 1

    nc.finalize()
    return nc


def _host_prep(voxel_features, coords, map_fm):
    """Shard points by core, build fp16 feature/pos tables (index work)."""
    vf = np.asarray(voxel_features)
    cd = np.asarray(coords)
    mf = np.asarray(map_fm)
    if mf.ndim == 5:
        mf = np.squeeze(mf, 3)

    b = cd[:, 0].astype(np.int64)
    y = cd[:, 2].astype(np.int64)
    x = cd[:, 3].astype(np.int64)
    valid = (b >= 0) & (b < B) & (y >= 0) & (y < NY) & (x >= 0) & (x < NX)
    b, y, x = b[valid], y[valid], x[valid]
    vfv = np.ascontiguousarray(vf[valid]).astype(np.float16)

    half = (y >= NYH).astype(np.int64)
    core = b * 2 + half
    lcell = (y - half * NYH) * NX + x
    t = lcell // TILE          # 512-cell tile id
    pos = lcell - t * TILE     # position within tile (= matmul column)
    pair = t % NP              # tile j pairs with tile j+NP
    blk = t // NP              # channel block within the pair

    key = core * NP + pair
    order = np.argsort(key, kind="stable")
    ks = key[order]
    counts = np.bincount(ks, minlength=NCORE * NP)
    kmax = counts.reshape(NCORE, NP).max(axis=0)
    chunks = np.maximum((kmax + CAP - 1) // CAP, 1)
    for g in range(0, NP, SG):
        need = int(chunks[g:g + SG].sum())
        if need > FBMAX:
            raise ValueError("pair group needs %d cols > FBMAX=%d"
                             % (need, FBMAX))
    ncols = int(chunks.sum())
    colbase = np.concatenate([[0], np.cumsum(chunks)]).astype(np.int64)

    starts = np.concatenate([[0], np.cumsum(counts)]).astype(np.int64)
    rank = np.arange(len(ks), dtype=np.int64) - starts[ks]

    co = core[order]
    po = pair[order]
    bo = blk[order]
    colo = colbase[po] + rank // CAP
    slot = rank % CAP

    feat = np.zeros((NCORE, CAP, ncols, 2 * C), np.float16)
    post = np.full((NCORE, CAP, ncols), -1.0, np.float32)
    ccol = bo[:, None] * C + np.arange(C)[None, :]
    feat[co[:, None], slot[:, None], colo[:, None], ccol] = vfv[order]
    post[co, slot, colo] = pos[order].astype(np.float32)

    maps = []
    for core_id in range(NCORE):
        bb, hh = core_id // 2, core_id % 2
        maps.append(np.ascontiguousarray(
            mf[bb, :, hh * NYH:(hh + 1) * NYH, :]).astype(
                np.float16).reshape(NX, NYH * CM))
    return feat, post, maps, ncols, chunks


def kernel(voxel_features, coords, batch_size=None, map_fm=None,
           trace=False, _return_results=False):
    from concourse.bass_utils import run_bass_kernel_spmd

    feat, post, maps, ncols, chunks = _host_prep(
        voxel_features, coords, map_fm)

    ckey = (ncols, tuple(int(c) for c in chunks))
    if ckey not in _prog_cache:
        _prog_cache.clear()
        _prog_cache[ckey] = _build_program(ncols, chunks)
    nc = _prog_cache[ckey]

    in_maps = [
        {"feat": feat[i].reshape(CAP, -1), "post": post[i],
         "mapin": maps[i]}
        for i in range(NCORE)
    ]
    res = run_bass_kernel_spmd(nc, in_maps, list(range(NCORE)), trace=trace)

    out = np.empty((B, C + CM, NY, NX), np.float32)
    for core_id in range(NCORE):
        bb, hh = core_id // 2, core_id % 2
        canv = res.results[core_id]["canv"]          # [128, ACELL] f16
        full = np.concatenate(
            [canv[0:C], canv[C:, :BCELL]], axis=1).astype(np.float32)
        out[bb, :C, hh * NYH:(hh + 1) * NYH, :] = full.reshape(C, NYH, NX)
        mo = res.results[core_id]["mapo"]            # [128, NYB*NX] f16
        out[bb, C:, hh * NYH:(hh + 1) * NYH, :] = (
            mo.reshape(YB, CM, NYB, NX).transpose(1, 2, 0, 3)
            .astype(np.float32).reshape(CM, NYH, NX))
    if _return_results:
        return out, res
    return out
